# revision 37
# baseline (speedup 1.0000x reference)
"""Trainium2 Bass kernel for GQA MultiHeadAttention with RoPE.

Shapes (hardcoded): x (2,2048,1024), Wq (1024,1024), Wk/Wv (1024,256),
Wo (1024,1024). 16 q-heads, 4 kv-heads, head_dim 64.

Sharding: 8 cores = batch (2) x kv-group (4). Core i handles b=i//4,
g=i%4, q-heads {g, 4+g, 8+g, 12+g} (jnp.tile GQA mapping), kv-head g.
Each core emits a partial Y^T (1024,2048); the host sums the 4 group
partials per batch and transposes.

Faithful to the reference's multiplicative tril mask before softmax:
  P = exp(mask * (Q K^T) * D**-0.5)   (masked entries = exp(0) = 1)
  out = (P @ V_aug) / Z,  Z carried in V_aug's ones column; fully-masked
  future tiles enter analytically via suffix sums of V (weight exp(0)=1
  per masked position) fused into the normalize pass.

Matmul operands are bf16 (fp32 PSUM accumulation); set DTMM to
mybir.dt.float32r for a ~3x slower, ~6x more accurate fallback.
"""

import os
import numpy as np
import ml_dtypes

import concourse.bass as bass
import concourse.mybir as mybir
import concourse.tile as tile
from concourse.masks import make_identity
from concourse.bass_utils import run_bass_kernel_spmd

F32 = mybir.dt.float32
DTMM = mybir.dt.bfloat16          # matmul operand dtype
NPMM = ml_dtypes.bfloat16
EXP = mybir.ActivationFunctionType.Exp

B, T, C = 2, 2048, 1024
NH, NKV, D = 16, 4, 64
HG = NH // NKV            # 4 q-heads per kv-group
NQ = 512                  # tq chunk width
NCH = T // NQ             # 4 chunks
NKT = T // 128            # 16 tk tiles
SCALE = D ** -0.5


def _split_waits(nc, max_waits=1):
    """This walrus build accepts only one immediate sem-wait per
    instruction; move extras onto preceding same-engine NoOps."""
    for f in nc.m.functions:
        for blk in f.blocks:
            new_insts = []
            for ins in blk.instructions:
                si = ins.sync_info
                if si is not None and len(si.on_wait) > max_waits:
                    waits = list(si.on_wait)
                    extra, keep = waits[:-max_waits], waits[-max_waits:]
                    k = 0
                    while extra:
                        chunk, extra = extra[:max_waits], extra[max_waits:]
                        nop = mybir.InstNoOp(name=f"{ins.name}-ws{k}", ins=[], outs=[])
                        nop.engine = ins.engine
                        nop.sync_info = mybir.SyncInfo(on_wait=chunk, on_update=[])
                        new_insts.append(nop)
                        k += 1
                    si.on_wait = keep
                new_insts.append(ins)
            blk.instructions[:] = new_insts


def _half_swap(nc, dst, src, base):
    """dst rows [base:base+64] = src rows [base+32:base+64],[base:base+32]."""
    nc.gpsimd.dma_start(out=dst[base:base + 32, :], in_=src[base + 32:base + 64, :])
    nc.gpsimd.dma_start(out=dst[base + 32:base + 64, :], in_=src[base:base + 32, :])


def _emit(nc, tc, ctx, xT, wq, wkv, wo, ctab, stab, mtab, yT):
    # ---------- whole-kernel SBUF ----------
    poolW = ctx.enter_context(tc.tile_pool(name="poolW", bufs=1))
    qrot = [poolW.tile([128, T], DTMM, tag=f"qrot{p}", name=f"qrot{p}")
            for p in range(2)]
    krot2 = poolW.tile([128, T], DTMM)      # K dup'd; V parked in rows 64-127 early
    vaug = poolW.tile([128, NKT * 65], DTMM)
    maskt = poolW.tile([128, 4 * NQ], F32)
    ostk = [poolW.tile([128, T], DTMM, tag=f"ostk{p}", name=f"ostk{p}")
            for p in range(2)]              # rows 0-63 head 2p, 64-127 head 2p+1
    sfcol = poolW.tile([64, 4], F32)        # suffix sums of V (col 3 = 0)
    smallf = poolW.tile([128, NQ + 64], F32)   # [0:NQ) ones, [NQ:NQ+64) identity
    smallr = poolW.tile([128, NQ + 64], DTMM)
    IDR = smallr[:, NQ: NQ + 64]

    nc.vector.memset(smallf[:, 0:NQ], 1.0)
    nc.gpsimd.memset(smallf[:, NQ:NQ + 64], 0.0)
    make_identity(nc, smallf[0:64, NQ:NQ + 64], nomemset=True)
    nc.sync.dma_start(out=smallf[64:128, NQ:NQ + 64], in_=smallf[0:64, NQ:NQ + 64])
    with nc.allow_low_precision(reason="bf16 constants"):
        nc.vector.tensor_copy(smallr[:], smallf[:])
    nc.sync.dma_start(out=maskt[:], in_=mtab[:])

    # ---------- single-phase pools (no release barriers) ----------
    poolA = ctx.enter_context(tc.tile_pool(name="poolA", bufs=1))
    stg = ctx.enter_context(tc.tile_pool(name="stg", bufs=2))
    poolB = ctx.enter_context(tc.tile_pool(name="poolB", bufs=1))
    dramB = ctx.enter_context(tc.tile_pool(name="dramB", bufs=1, space="DRAM"))
    ps = ctx.enter_context(tc.tile_pool(name="ps", bufs=1, space="PSUM"))

    xtr = [poolA.tile([128, T], DTMM, tag=f"xtr{i}", name=f"xtr{i}")
           for i in range(8)]
    wqr = poolA.tile([128, 8 * 256], DTMM)
    wkvr = poolA.tile([128, 8 * 128], DTMM)
    cost = poolA.tile([128, T], F32)
    sint = poolA.tile([128, T], F32)
    wor = [poolB.tile([128, C], DTMM, tag=f"wor{p}", name=f"wor{p}")
           for p in range(2)]
    # x chunk 0 + wkv first so the first projection starts ASAP
    for i in range(8):
        nc.sync.dma_start(out=xtr[i][:, 0:NQ], in_=xT[i * 128:(i + 1) * 128, 0:NQ])
    for i in range(8):
        nc.sync.dma_start(out=wkvr[:, i * 128:(i + 1) * 128],
                          in_=wkv[i * 128:(i + 1) * 128, :])
    for tcx in range(1, NCH):
        sl = slice(tcx * NQ, (tcx + 1) * NQ)
        for i in range(8):
            nc.sync.dma_start(out=xtr[i][:, sl], in_=xT[i * 128:(i + 1) * 128, sl])
    nc.sync.dma_start(out=cost[:], in_=ctab[:])
    nc.sync.dma_start(out=sint[:], in_=stab[:])
    for i in range(8):
        nc.sync.dma_start(out=wqr[:, i * 256:(i + 1) * 256],
                          in_=wq[i * 128:(i + 1) * 128, :])
    for p in range(2):
        nc.sync.dma_start(out=wor[p][:], in_=wo[p * 128:(p + 1) * 128, :])

    # K/V projection; K RoPE into krot2[0:64], V parked in krot2[64:128]
    for tcx in range(NCH):
        sl = slice(tcx * NQ, (tcx + 1) * NQ)
        kvps = ps.tile([128, 2 * NQ], F32, tag="mm2b", bufs=3)
        for i in range(8):
            nc.tensor.matmul(kvps[:, 0:NQ], wkvr[:, i * 128:(i + 1) * 128],
                             xtr[i][:, sl], start=(i == 0), stop=(i == 7))
        kcp = stg.tile([128, NQ], F32, tag="pcp")
        nc.vector.tensor_copy(kcp[0:64, :], kvps[0:64, 0:NQ])
        swp = stg.tile([128, NQ], F32, tag="swp")
        _half_swap(nc, swp, kcp, 0)
        t1 = stg.tile([128, NQ], F32, tag="t1")
        t2 = stg.tile([128, NQ], F32, tag="t2")
        nc.vector.tensor_mul(t1[0:64, :], kcp[0:64, :], cost[0:64, sl])
        nc.vector.tensor_mul(t2[0:64, :], swp[0:64, :], sint[0:64, sl])
        with nc.allow_low_precision(reason="bf16 K"):
            nc.vector.tensor_add(krot2[0:64, sl], t1[0:64, :], t2[0:64, :])
            nc.vector.tensor_copy(krot2[64:128, sl], kvps[64:128, 0:NQ])

    # V transpose into vaug (+ ones column)
    for kt in range(NKT):
        vtp = ps.tile([128, 64], DTMM, tag="ops", bufs=1)
        with nc.allow_low_precision(reason="bf16 PE transpose of V"):
            nc.tensor.transpose(vtp[:], krot2[64:128, kt * 128:(kt + 1) * 128],
                                IDR[64:128, :])
            nc.vector.tensor_copy(vaug[:, kt * 65:kt * 65 + 64], vtp[:])
            nc.vector.tensor_copy(vaug[:, kt * 65 + 64:kt * 65 + 65],
                                  smallr[:, 0:1])
    # suffix sums of V^T along t (for the analytic future-tile term)
    redc = poolA.tile([128, 4], F32)
    nc.gpsimd.memset(redc[:], 0.0)
    for c in range(NCH - 1):
        nc.vector.tensor_reduce(redc[64:128, c:c + 1],
                                krot2[64:128, (c + 1) * NQ:T],
                                axis=mybir.AxisListType.X,
                                op=mybir.AluOpType.add)
    nc.gpsimd.dma_start(out=sfcol[:], in_=redc[64:128, :])
    nc.gpsimd.dma_start(out=krot2[64:128, :], in_=krot2[0:64, :])

    def emit_qproj(tcx):
        for p in range(2):
            sl = slice(tcx * NQ, (tcx + 1) * NQ)
            qps = ps.tile([128, 2 * NQ], F32, tag="mm2b", bufs=3, name="qps")
            for i in range(8):
                nc.tensor.matmul(
                    qps[:, 0:NQ], wqr[:, i * 256 + p * 128: i * 256 + (p + 1) * 128],
                    xtr[i][:, sl], start=(i == 0), stop=(i == 7))
            qcp = stg.tile([128, NQ], F32, tag="pcp", name="qcp")
            nc.vector.tensor_copy(qcp[:], qps[:, 0:NQ])
            swp = stg.tile([128, NQ], F32, tag="swp", name="swp")
            _half_swap(nc, swp, qcp, 0)
            _half_swap(nc, swp, qcp, 64)
            t1 = stg.tile([128, NQ], F32, tag="t1", name="t1")
            t2 = stg.tile([128, NQ], F32, tag="t2", name="t2")
            nc.vector.tensor_mul(t1[:], qcp[:], cost[:, sl])
            nc.vector.tensor_mul(t2[:], swp[:], sint[:, sl])
            with nc.allow_low_precision(reason="bf16 Q"):
                nc.vector.tensor_add(qrot[p][:, sl], t1[:], t2[:])

    def emit_yproj(c):
        csl = slice(c * NQ, (c + 1) * NQ)
        for j in range(8):
            jsl = slice(j * 128, (j + 1) * 128)
            yps = ps.tile([128, NQ], F32, tag="ps1b", bufs=1, name="yps")
            for p in range(2):
                nc.tensor.matmul(yps[:], wor[p][:, jsl], ostk[p][:, csl],
                                 start=(p == 0), stop=(p == 1))
            ytmp = poolB.tile([128, NQ], F32, tag="ytmp", bufs=3, name="ytmp")
            nc.vector.tensor_copy(ytmp[:], yps[:])
            nc.sync.dma_start(out=yT[jsl, csl], in_=ytmp[:])

    # ---------- per-chunk: Q proj -> attention -> (deferred) out-proj ----------
    # Chunks descend so the largest attention chunk pipelines first and no
    # suffix-sum data is needed before it exists.
    for tcx in reversed(range(NCH)):
        emit_qproj(tcx)
    pending_y = None
    for c in reversed(range(NCH)):
        csl = slice(c * NQ, (c + 1) * NQ)
        if pending_y is not None:
            emit_yproj(pending_y)
        for h in range(HG):
            p, lo = h // 2, (h % 2) * 64
            hsl = slice(lo, lo + 64)
            ops = ps.tile([65, NQ], F32, tag="ops", bufs=1, name="ops")
            npair = 2 * (c + 1)
            nmm = 0
            LOOKAHEAD = 3
            pqs = {}
            for idx in range(npair + LOOKAHEAD):
                if idx < npair:     # emit S-pair(idx) + exp(idx)
                    q2 = idx
                    sq = ps.tile([128, 2 * NQ], F32, tag="mm2b", bufs=3, name="sq")
                    for i in range(2):
                        kt = 2 * q2 + i
                        nc.tensor.matmul(sq[:, i * NQ:(i + 1) * NQ],
                                         krot2[hsl, kt * 128:(kt + 1) * 128],
                                         qrot[p][hsl, csl], start=True, stop=True)
                    if q2 >= 2 * c:  # band pair: mask diagonal, zero above
                        for i in range(2):
                            kt = 2 * q2 + i
                            dlt = (kt - 4 * c) * 128
                            nc.vector.tensor_mul(
                                sq[:, i * NQ + dlt:i * NQ + dlt + 128],
                                sq[:, i * NQ + dlt:i * NQ + dlt + 128],
                                maskt[:, (kt - 4 * c) * NQ + dlt:
                                      (kt - 4 * c) * NQ + dlt + 128])
                            if dlt:
                                nc.vector.memset(sq[:, i * NQ:i * NQ + dlt], 0.0)
                    pq = poolB.tile([128, 2 * NQ], DTMM, tag="pquad", bufs=4,
                                    name="pq")
                    nc.scalar.activation(pq[:], sq[:], EXP, scale=SCALE)
                    pqs[q2] = pq
                if idx >= LOOKAHEAD:   # emit O-pair(idx - LOOKAHEAD)
                    q2 = idx - LOOKAHEAD
                    pq = pqs.pop(q2)
                    for i in range(2):
                        kt = 2 * q2 + i
                        nc.tensor.matmul(ops[:], vaug[:, kt * 65:(kt + 1) * 65],
                                         pq[:, i * NQ:(i + 1) * NQ],
                                         start=(nmm == 0),
                                         stop=(nmm == 2 * npair - 1))
                        nmm += 1
            # free the PSUM bank fast: copy O+Z to SBUF, then normalize
            ocp = poolB.tile([65, NQ], F32, tag="ocp", bufs=3, name="ocp")
            nc.vector.tensor_copy(ocp[:], ops[:])
            # Z += count of unprocessed positions (each exp(0)=1); spread Z
            # across 128 lanes, reciprocal, bounce via DRAM to broadcast
            cnt = float(T - (c + 1) * NQ)
            zsp = poolB.tile([128, 12], F32, tag="zsp", bufs=2, name="zsp")
            nc.gpsimd.dma_start(
                out=zsp[:, 0:4],
                in_=ocp[64:65, :].rearrange("p (a b) -> p a b", b=4))
            nc.vector.tensor_scalar_add(zsp[:, 4:8], zsp[:, 0:4], cnt)
            nc.vector.reciprocal(zsp[:, 8:12], zsp[:, 4:8])
            zdr = dramB.tile([1, NQ], F32, tag="zdr", bufs=2, name="zdr")
            nc.gpsimd.dma_start(
                out=zdr[:].rearrange("p (a b) -> p a b", b=4),
                in_=zsp[:, 8:12])
            rzb = poolB.tile([64, NQ], F32, tag="rzb", bufs=2, name="rzb")
            nc.gpsimd.dma_start(
                out=rzb[:],
                in_=bass.AP(tensor=zdr.tensor, offset=zdr.offset,
                            ap=[[0, 64]] + [zdr.ap[-1]]))
            # O = (P@V + suffixV) / Z
            with nc.allow_low_precision(reason="bf16 normalized O"):
                if h % 2 == 0:
                    nc.vector.scalar_tensor_tensor(
                        ostk[p][0:64, csl], ocp[0:64, :], sfcol[:, c:c + 1],
                        rzb[:], op0=mybir.AluOpType.add,
                        op1=mybir.AluOpType.mult)
                else:
                    otmp = poolB.tile([64, NQ], DTMM, tag="otmp", bufs=2,
                                      name="otmp")
                    nc.vector.scalar_tensor_tensor(
                        otmp[:], ocp[0:64, :], sfcol[:, c:c + 1],
                        rzb[:], op0=mybir.AluOpType.add,
                        op1=mybir.AluOpType.mult)
                    obn = dramB.tile([64, NQ], DTMM, tag="obn", bufs=2,
                                     name="obn")
                    nc.gpsimd.dma_start(out=obn[:], in_=otmp[:])
                    nc.gpsimd.dma_start(out=ostk[p][64:128, csl], in_=obn[:])
        pending_y = c
    emit_yproj(pending_y)


def _build(nrep=1):
    from contextlib import ExitStack
    nc = bass.Bass()
    xT = nc.declare_dram_parameter("xT", [C, T], DTMM, isOutput=False)
    wq = nc.declare_dram_parameter("wq", [C, HG * D], DTMM, isOutput=False)
    wkv = nc.declare_dram_parameter("wkv", [C, 2 * D], DTMM, isOutput=False)
    wo = nc.declare_dram_parameter("wo", [HG * D, C], DTMM, isOutput=False)
    ctab = nc.declare_dram_parameter("ctab", [128, T], F32, isOutput=False)
    stab = nc.declare_dram_parameter("stab", [128, T], F32, isOutput=False)
    mtab = nc.declare_dram_parameter("mtab", [128, 4 * NQ], F32, isOutput=False)
    yT = nc.declare_dram_parameter("yT", [C, T], F32, isOutput=True)

    with tile.TileContext(nc) as tc:
        for _ in range(nrep):
            with ExitStack() as ctx:
                _emit(nc, tc, ctx, xT, wq, wkv, wo, ctab, stab, mtab, yT)
    _split_waits(nc)
    return nc


def _host_inputs(x, Wq, Wk, Wv, Wo):
    perm = np.concatenate([np.arange(0, D, 2), np.arange(1, D, 2)])  # even-first
    inv_freq = 1.0 / (10000.0 ** (np.arange(0, D, 2, dtype=np.float64) / D))
    ang = np.arange(T, dtype=np.float64)[:, None] * inv_freq[None, :]
    cos = np.cos(ang).astype(np.float32).T      # (32, T)
    sin = np.sin(ang).astype(np.float32).T
    ctab = np.ascontiguousarray(np.tile(cos, (4, 1)))                 # (128, T)
    stab = np.ascontiguousarray(np.concatenate([-sin, sin, -sin, sin], 0))
    f = np.arange(NQ)[None, :]
    pcol = np.arange(128)[:, None]
    mtab = np.ascontiguousarray(np.concatenate(
        [(pcol + i * 128 <= f).astype(np.float32) for i in range(4)], axis=1))

    xTb = [np.ascontiguousarray(x[b].T.astype(NPMM)) for b in range(B)]
    maps = []
    for core in range(8):
        b, g = core // 4, core % 4
        heads = [g + NKV * k for k in range(HG)]
        wq_cols = np.concatenate([h * D + perm for h in heads])
        wq_g = np.ascontiguousarray(Wq[:, wq_cols].astype(NPMM))
        wkv_g = np.ascontiguousarray(np.concatenate(
            [Wk[:, g * D + perm], Wv[:, g * D:(g + 1) * D]], axis=1).astype(NPMM))
        wo_rows = np.concatenate([np.arange(h * D, (h + 1) * D) for h in heads])
        wo_g = np.ascontiguousarray(Wo[wo_rows, :].astype(NPMM))
        maps.append({"xT": xTb[b], "wq": wq_g, "wkv": wkv_g, "wo": wo_g,
                     "ctab": ctab, "stab": stab, "mtab": mtab})
    return maps


_CACHE = {}


def kernel(x, Wq, Wk, Wv, Wo):
    if "nc" not in _CACHE:
        _CACHE["nc"] = _build()
    nc = _CACHE["nc"]
    maps = _host_inputs(np.asarray(x, np.float32), np.asarray(Wq, np.float32),
                        np.asarray(Wk, np.float32), np.asarray(Wv, np.float32),
                        np.asarray(Wo, np.float32))
    trace = bool(int(os.environ.get("BASSKERNEL_TRACE", "0")))
    res = run_bass_kernel_spmd(nc, maps, list(range(8)), trace=trace)
    if trace and res.exec_time_ns is not None:
        print(f"HW exec time: {res.exec_time_ns} ns")
    out = np.zeros((B, T, C), dtype=np.float32)
    for core in range(8):
        out[core // 4] += res.results[core]["yT"].T
    return out


# revision 38
# speedup vs baseline: 1.1399x; 1.1399x over previous
"""Trainium2 Bass kernel for GQA MultiHeadAttention with RoPE.

Shapes (hardcoded): x (2,2048,1024), Wq (1024,1024), Wk/Wv (1024,256),
Wo (1024,1024). 16 q-heads, 4 kv-heads, head_dim 64.

Sharding: 8 cores = batch (2) x kv-group (4). Core i handles b=i//4,
g=i%4, q-heads {g, 4+g, 8+g, 12+g} (jnp.tile GQA mapping), kv-head g.
Each core emits a partial Y^T (1024,2048); the host sums the 4 group
partials per batch and transposes.

Faithful to the reference's multiplicative tril mask before softmax:
  P = exp(mask * (Q K^T) * D**-0.5)   (masked entries = exp(0) = 1)
  out = (P @ V_aug) / Z,  Z carried in V_aug's ones column; fully-masked
  future tiles enter analytically via suffix sums of V (weight exp(0)=1
  per masked position) fused into the normalize pass.

Matmul operands are bf16 (fp32 PSUM accumulation); set DTMM to
mybir.dt.float32r for a ~3x slower, ~6x more accurate fallback.
"""

import os
import numpy as np
import ml_dtypes

import concourse.bass as bass
import concourse.mybir as mybir
import concourse.tile as tile
from concourse.masks import make_identity
from concourse.bass_utils import run_bass_kernel_spmd

F32 = mybir.dt.float32
DTMM = mybir.dt.bfloat16          # matmul operand dtype
NPMM = ml_dtypes.bfloat16
EXP = mybir.ActivationFunctionType.Exp

B, T, C = 2, 2048, 1024
NH, NKV, D = 16, 4, 64
HG = NH // NKV            # 4 q-heads per kv-group
NQ = 512                  # tq chunk width
NCH = T // NQ             # 4 chunks
NKT = T // 128            # 16 tk tiles
SCALE = D ** -0.5


def _split_waits(nc, max_waits=1):
    """This walrus build accepts only one immediate sem-wait per
    instruction; move extras onto preceding same-engine NoOps."""
    for f in nc.m.functions:
        for blk in f.blocks:
            new_insts = []
            for ins in blk.instructions:
                si = ins.sync_info
                if si is not None and len(si.on_wait) > max_waits:
                    waits = list(si.on_wait)
                    extra, keep = waits[:-max_waits], waits[-max_waits:]
                    k = 0
                    while extra:
                        chunk, extra = extra[:max_waits], extra[max_waits:]
                        nop = mybir.InstNoOp(name=f"{ins.name}-ws{k}", ins=[], outs=[])
                        nop.engine = ins.engine
                        nop.sync_info = mybir.SyncInfo(on_wait=chunk, on_update=[])
                        new_insts.append(nop)
                        k += 1
                    si.on_wait = keep
                new_insts.append(ins)
            blk.instructions[:] = new_insts


def _half_swap(nc, dst, src, base):
    """dst rows [base:base+64] = src rows [base+32:base+64],[base:base+32]."""
    nc.gpsimd.dma_start(out=dst[base:base + 32, :], in_=src[base + 32:base + 64, :])
    nc.gpsimd.dma_start(out=dst[base + 32:base + 64, :], in_=src[base:base + 32, :])


def _emit(nc, tc, ctx, xT, wq, wkv, wo, ctab, stab, mtab, yT):
    # ---------- whole-kernel SBUF ----------
    poolW = ctx.enter_context(tc.tile_pool(name="poolW", bufs=1))
    qrot = [poolW.tile([128, T], DTMM, tag=f"qrot{p}", name=f"qrot{p}")
            for p in range(2)]
    krot2 = poolW.tile([128, T], DTMM)      # K dup'd; V parked in rows 64-127 early
    vaug = poolW.tile([128, NKT * 65], DTMM)
    maskt = poolW.tile([128, 4 * NQ], F32)
    ostk = [poolW.tile([128, T], DTMM, tag=f"ostk{p}", name=f"ostk{p}")
            for p in range(2)]              # rows 0-63 head 2p, 64-127 head 2p+1
    sfcol = poolW.tile([64, 4], F32)        # suffix sums of V (col 3 = 0)
    smallf = poolW.tile([128, NQ + 64], F32)   # [0:NQ) ones, [NQ:NQ+64) identity
    smallr = poolW.tile([128, NQ + 64], DTMM)
    IDR = smallr[:, NQ: NQ + 64]

    nc.vector.memset(smallf[:, 0:NQ], 1.0)
    nc.gpsimd.memset(smallf[:, NQ:NQ + 64], 0.0)
    make_identity(nc, smallf[0:64, NQ:NQ + 64], nomemset=True)
    nc.sync.dma_start(out=smallf[64:128, NQ:NQ + 64], in_=smallf[0:64, NQ:NQ + 64])
    with nc.allow_low_precision(reason="bf16 constants"):
        nc.vector.tensor_copy(smallr[:], smallf[:])
    nc.sync.dma_start(out=maskt[:], in_=mtab[:])

    # ---------- single-phase pools (no release barriers) ----------
    poolA = ctx.enter_context(tc.tile_pool(name="poolA", bufs=1))
    stg = ctx.enter_context(tc.tile_pool(name="stg", bufs=2))
    poolB = ctx.enter_context(tc.tile_pool(name="poolB", bufs=1))
    dramB = ctx.enter_context(tc.tile_pool(name="dramB", bufs=1, space="DRAM"))
    ps = ctx.enter_context(tc.tile_pool(name="ps", bufs=1, space="PSUM"))

    xtr = [poolA.tile([128, T], DTMM, tag=f"xtr{i}", name=f"xtr{i}")
           for i in range(8)]
    wqr = poolA.tile([128, 8 * 256], DTMM)
    wkvr = poolA.tile([128, 8 * 128], DTMM)
    cost = poolA.tile([128, T], F32)
    sint = poolA.tile([128, T], F32)
    wor = [poolB.tile([128, C], DTMM, tag=f"wor{p}", name=f"wor{p}")
           for p in range(2)]
    # x chunk 0 + wkv first so the first projection starts ASAP
    for i in range(8):
        nc.sync.dma_start(out=xtr[i][:, 0:NQ], in_=xT[i * 128:(i + 1) * 128, 0:NQ])
    for i in range(8):
        nc.sync.dma_start(out=wkvr[:, i * 128:(i + 1) * 128],
                          in_=wkv[i * 128:(i + 1) * 128, :])
    for tcx in range(1, NCH):
        sl = slice(tcx * NQ, (tcx + 1) * NQ)
        for i in range(8):
            nc.sync.dma_start(out=xtr[i][:, sl], in_=xT[i * 128:(i + 1) * 128, sl])
    nc.sync.dma_start(out=cost[:], in_=ctab[:])
    nc.sync.dma_start(out=sint[:], in_=stab[:])
    for i in range(8):
        nc.sync.dma_start(out=wqr[:, i * 256:(i + 1) * 256],
                          in_=wq[i * 128:(i + 1) * 128, :])
    for p in range(2):
        nc.sync.dma_start(out=wor[p][:], in_=wo[p * 128:(p + 1) * 128, :])

    # K/V projection; K RoPE into krot2[0:64], V parked in krot2[64:128]
    for tcx in range(NCH):
        sl = slice(tcx * NQ, (tcx + 1) * NQ)
        kvps = ps.tile([128, 2 * NQ], F32, tag="mm2b", bufs=3)
        for i in range(8):
            nc.tensor.matmul(kvps[:, 0:NQ], wkvr[:, i * 128:(i + 1) * 128],
                             xtr[i][:, sl], start=(i == 0), stop=(i == 7))
        kcp = stg.tile([128, NQ], F32, tag="pcp")
        nc.vector.tensor_copy(kcp[0:64, :], kvps[0:64, 0:NQ])
        swp = stg.tile([128, NQ], F32, tag="swp")
        _half_swap(nc, swp, kcp, 0)
        t1 = stg.tile([128, NQ], F32, tag="t1")
        t2 = stg.tile([128, NQ], F32, tag="t2")
        nc.vector.tensor_mul(t1[0:64, :], kcp[0:64, :], cost[0:64, sl])
        nc.vector.tensor_mul(t2[0:64, :], swp[0:64, :], sint[0:64, sl])
        with nc.allow_low_precision(reason="bf16 K"):
            nc.vector.tensor_add(krot2[0:64, sl], t1[0:64, :], t2[0:64, :])
            nc.vector.tensor_copy(krot2[64:128, sl], kvps[64:128, 0:NQ])

    # V transpose into vaug (+ ones column)
    for kt in range(NKT):
        vtp = ps.tile([128, 64], DTMM, tag="ops", bufs=1)
        with nc.allow_low_precision(reason="bf16 PE transpose of V"):
            nc.tensor.transpose(vtp[:], krot2[64:128, kt * 128:(kt + 1) * 128],
                                IDR[64:128, :])
            nc.vector.tensor_copy(vaug[:, kt * 65:kt * 65 + 64], vtp[:])
            nc.vector.tensor_copy(vaug[:, kt * 65 + 64:kt * 65 + 65],
                                  smallr[:, 0:1])
    # suffix sums of V^T along t (for the analytic future-tile term)
    redc = poolA.tile([128, 4], F32)
    nc.gpsimd.memset(redc[:], 0.0)
    for c in range(NCH - 1):
        nc.vector.tensor_reduce(redc[64:128, c:c + 1],
                                krot2[64:128, (c + 1) * NQ:T],
                                axis=mybir.AxisListType.X,
                                op=mybir.AluOpType.add)
    nc.gpsimd.dma_start(out=sfcol[:], in_=redc[64:128, :])
    nc.gpsimd.dma_start(out=krot2[64:128, :], in_=krot2[0:64, :])

    def emit_qproj(tcx):
        for p in range(2):
            sl = slice(tcx * NQ, (tcx + 1) * NQ)
            qps = ps.tile([128, 2 * NQ], F32, tag="mm2b", bufs=3, name="qps")
            for i in range(8):
                nc.tensor.matmul(
                    qps[:, 0:NQ], wqr[:, i * 256 + p * 128: i * 256 + (p + 1) * 128],
                    xtr[i][:, sl], start=(i == 0), stop=(i == 7))
            qcp = stg.tile([128, NQ], F32, tag="pcp", name="qcp")
            nc.vector.tensor_copy(qcp[:], qps[:, 0:NQ])
            swp = stg.tile([128, NQ], F32, tag="swp", name="swp")
            _half_swap(nc, swp, qcp, 0)
            _half_swap(nc, swp, qcp, 64)
            t1 = stg.tile([128, NQ], F32, tag="t1", name="t1")
            t2 = stg.tile([128, NQ], F32, tag="t2", name="t2")
            nc.vector.tensor_mul(t1[:], qcp[:], cost[:, sl])
            nc.vector.tensor_mul(t2[:], swp[:], sint[:, sl])
            with nc.allow_low_precision(reason="bf16 Q"):
                nc.vector.tensor_add(qrot[p][:, sl], t1[:], t2[:])

    def emit_yproj(c):
        csl = slice(c * NQ, (c + 1) * NQ)
        for j in range(8):
            jsl = slice(j * 128, (j + 1) * 128)
            yps = ps.tile([128, NQ], F32, tag="ps1b", bufs=1, name="yps")
            for p in range(2):
                nc.tensor.matmul(yps[:], wor[p][:, jsl], ostk[p][:, csl],
                                 start=(p == 0), stop=(p == 1))
            ytmp = poolB.tile([128, NQ], F32, tag="ytmp", bufs=3, name="ytmp")
            nc.vector.tensor_copy(ytmp[:], yps[:])
            nc.sync.dma_start(out=yT[jsl, csl], in_=ytmp[:])

    # ---------- per-chunk: Q proj -> attention -> (deferred) out-proj ----------
    # Chunks descend so the largest attention chunk pipelines first and no
    # suffix-sum data is needed before it exists.
    for tcx in reversed(range(NCH)):
        emit_qproj(tcx)
    pending_y = None
    for c in reversed(range(NCH)):
        csl = slice(c * NQ, (c + 1) * NQ)
        if pending_y is not None:
            emit_yproj(pending_y)
        for h in range(HG):
            p, lo = h // 2, (h % 2) * 64
            hsl = slice(lo, lo + 64)
            ops = ps.tile([65, NQ], F32, tag="ops", bufs=1, name="ops")
            npair = 2 * (c + 1)
            nmm = 0
            LOOKAHEAD = 2
            pqs = {}
            for idx in range(npair + LOOKAHEAD):
                if idx < npair:     # emit S-pair(idx) + exp(idx)
                    q2 = idx
                    sq = ps.tile([128, 2 * NQ], F32, tag="mm2b", bufs=3, name="sq")
                    for i in range(2):
                        kt = 2 * q2 + i
                        nc.tensor.matmul(sq[:, i * NQ:(i + 1) * NQ],
                                         krot2[hsl, kt * 128:(kt + 1) * 128],
                                         qrot[p][hsl, csl], start=True, stop=True)
                    if q2 >= 2 * c:  # band pair: mask diagonal, zero above
                        for i in range(2):
                            kt = 2 * q2 + i
                            dlt = (kt - 4 * c) * 128
                            nc.vector.tensor_mul(
                                sq[:, i * NQ + dlt:i * NQ + dlt + 128],
                                sq[:, i * NQ + dlt:i * NQ + dlt + 128],
                                maskt[:, (kt - 4 * c) * NQ + dlt:
                                      (kt - 4 * c) * NQ + dlt + 128])
                            if dlt:
                                nc.vector.memset(sq[:, i * NQ:i * NQ + dlt], 0.0)
                    pq = poolB.tile([128, 2 * NQ], DTMM, tag="pquad", bufs=4,
                                    name="pq")
                    nc.scalar.activation(pq[:], sq[:], EXP, scale=SCALE)
                    pqs[q2] = pq
                if idx >= LOOKAHEAD:   # emit O-pair(idx - LOOKAHEAD)
                    q2 = idx - LOOKAHEAD
                    pq = pqs.pop(q2)
                    for i in range(2):
                        kt = 2 * q2 + i
                        nc.tensor.matmul(ops[:], vaug[:, kt * 65:(kt + 1) * 65],
                                         pq[:, i * NQ:(i + 1) * NQ],
                                         start=(nmm == 0),
                                         stop=(nmm == 2 * npair - 1))
                        nmm += 1
            # free the PSUM bank fast: copy O+Z to SBUF, then normalize
            ocp = poolB.tile([65, NQ], F32, tag="ocp", bufs=3, name="ocp")
            nc.vector.tensor_copy(ocp[:], ops[:])
            # Z += count of unprocessed positions (each exp(0)=1); spread Z
            # across 128 lanes, reciprocal, bounce via DRAM to broadcast
            cnt = float(T - (c + 1) * NQ)
            zsp = poolB.tile([128, 12], F32, tag="zsp", bufs=2, name="zsp")
            nc.gpsimd.dma_start(
                out=zsp[:, 0:4],
                in_=ocp[64:65, :].rearrange("p (a b) -> p a b", b=4))
            nc.vector.tensor_scalar_add(zsp[:, 4:8], zsp[:, 0:4], cnt)
            nc.vector.reciprocal(zsp[:, 8:12], zsp[:, 4:8])
            zdr = dramB.tile([1, NQ], F32, tag="zdr", bufs=2, name="zdr")
            nc.gpsimd.dma_start(
                out=zdr[:].rearrange("p (a b) -> p a b", b=4),
                in_=zsp[:, 8:12])
            rzb = poolB.tile([64, NQ], F32, tag="rzb", bufs=2, name="rzb")
            nc.gpsimd.dma_start(
                out=rzb[:],
                in_=bass.AP(tensor=zdr.tensor, offset=zdr.offset,
                            ap=[[0, 64]] + [zdr.ap[-1]]))
            # O = (P@V + suffixV) / Z
            with nc.allow_low_precision(reason="bf16 normalized O"):
                if h % 2 == 0:
                    nc.vector.scalar_tensor_tensor(
                        ostk[p][0:64, csl], ocp[0:64, :], sfcol[:, c:c + 1],
                        rzb[:], op0=mybir.AluOpType.add,
                        op1=mybir.AluOpType.mult)
                else:
                    otmp = poolB.tile([64, NQ], DTMM, tag="otmp", bufs=2,
                                      name="otmp")
                    nc.vector.scalar_tensor_tensor(
                        otmp[:], ocp[0:64, :], sfcol[:, c:c + 1],
                        rzb[:], op0=mybir.AluOpType.add,
                        op1=mybir.AluOpType.mult)
                    obn = dramB.tile([64, NQ], DTMM, tag="obn", bufs=2,
                                     name="obn")
                    nc.gpsimd.dma_start(out=obn[:], in_=otmp[:])
                    nc.gpsimd.dma_start(out=ostk[p][64:128, csl], in_=obn[:])
        pending_y = c
    emit_yproj(pending_y)


def _build(nrep=1):
    from contextlib import ExitStack
    nc = bass.Bass()
    xT = nc.declare_dram_parameter("xT", [C, T], DTMM, isOutput=False)
    wq = nc.declare_dram_parameter("wq", [C, HG * D], DTMM, isOutput=False)
    wkv = nc.declare_dram_parameter("wkv", [C, 2 * D], DTMM, isOutput=False)
    wo = nc.declare_dram_parameter("wo", [HG * D, C], DTMM, isOutput=False)
    ctab = nc.declare_dram_parameter("ctab", [128, T], F32, isOutput=False)
    stab = nc.declare_dram_parameter("stab", [128, T], F32, isOutput=False)
    mtab = nc.declare_dram_parameter("mtab", [128, 4 * NQ], F32, isOutput=False)
    yT = nc.declare_dram_parameter("yT", [C, T], F32, isOutput=True)

    with tile.TileContext(nc) as tc:
        for _ in range(nrep):
            with ExitStack() as ctx:
                _emit(nc, tc, ctx, xT, wq, wkv, wo, ctab, stab, mtab, yT)
    _split_waits(nc)
    return nc


def _host_inputs(x, Wq, Wk, Wv, Wo):
    perm = np.concatenate([np.arange(0, D, 2), np.arange(1, D, 2)])  # even-first
    inv_freq = 1.0 / (10000.0 ** (np.arange(0, D, 2, dtype=np.float64) / D))
    ang = np.arange(T, dtype=np.float64)[:, None] * inv_freq[None, :]
    cos = np.cos(ang).astype(np.float32).T      # (32, T)
    sin = np.sin(ang).astype(np.float32).T
    ctab = np.ascontiguousarray(np.tile(cos, (4, 1)))                 # (128, T)
    stab = np.ascontiguousarray(np.concatenate([-sin, sin, -sin, sin], 0))
    f = np.arange(NQ)[None, :]
    pcol = np.arange(128)[:, None]
    mtab = np.ascontiguousarray(np.concatenate(
        [(pcol + i * 128 <= f).astype(np.float32) for i in range(4)], axis=1))

    xTb = [np.ascontiguousarray(x[b].T.astype(NPMM)) for b in range(B)]
    maps = []
    for core in range(8):
        b, g = core // 4, core % 4
        heads = [g + NKV * k for k in range(HG)]
        wq_cols = np.concatenate([h * D + perm for h in heads])
        wq_g = np.ascontiguousarray(Wq[:, wq_cols].astype(NPMM))
        wkv_g = np.ascontiguousarray(np.concatenate(
            [Wk[:, g * D + perm], Wv[:, g * D:(g + 1) * D]], axis=1).astype(NPMM))
        wo_rows = np.concatenate([np.arange(h * D, (h + 1) * D) for h in heads])
        wo_g = np.ascontiguousarray(Wo[wo_rows, :].astype(NPMM))
        maps.append({"xT": xTb[b], "wq": wq_g, "wkv": wkv_g, "wo": wo_g,
                     "ctab": ctab, "stab": stab, "mtab": mtab})
    return maps


_CACHE = {}


def kernel(x, Wq, Wk, Wv, Wo):
    if "nc" not in _CACHE:
        _CACHE["nc"] = _build()
    nc = _CACHE["nc"]
    maps = _host_inputs(np.asarray(x, np.float32), np.asarray(Wq, np.float32),
                        np.asarray(Wk, np.float32), np.asarray(Wv, np.float32),
                        np.asarray(Wo, np.float32))
    trace = bool(int(os.environ.get("BASSKERNEL_TRACE", "0")))
    res = run_bass_kernel_spmd(nc, maps, list(range(8)), trace=trace)
    if trace and res.exec_time_ns is not None:
        print(f"HW exec time: {res.exec_time_ns} ns")
    out = np.zeros((B, T, C), dtype=np.float32)
    for core in range(8):
        out[core // 4] += res.results[core]["yT"].T
    return out
